# revision 11
# baseline (speedup 1.0000x reference)
"""Banded (sliding-window) causal multi-head attention for Trainium2.

Problem: B=1, H=16, S=2048, DK=64 fp32; layer_idx=1 -> causal mask AND
(i - j) < 256 sliding window.  Returns (context, k, v) like the reference.

Sharding: 16 heads over 8 cores = 2 heads/core (pure head parallelism, no
inter-core communication).

Per-core algorithm (per head):
  - Host pre-transposes Q^T (scaled by 1/sqrt(dk)) and K^T to [64, S] fp16 and
    builds V_aug = [V | ones] ([S, 65]) fp16.  The ones column makes the PV
    matmul produce the softmax denominator for free.
  - For each 128-wide key block kb, compute st = K_kb @ Q^T over the query
    span that attends to kb (<= 384 wide): one PE matmul -> PSUM [128k, w].
    Two consecutive key blocks share one [128, 1024] PSUM pair-tile (one st in
    each 2KB bank) so a single ACT exp / DVE mask pass covers both.
  - ACT exp (no max subtraction: scores are O(+-8), safe in fp32/fp16) writes
    P^T in fp16; DVE multiplies by a static 0/1 band mask to zero invalid
    (k > q or q - k >= WIN) entries.
  - PV: ctx[qb] = sum_kb P^T_kb[:, qb-slice].T @ V_aug_kb accumulated in PSUM.
    Four query blocks share one PSUM bank; only the first matmul of the bank
    uses start=True (start zeroes the whole 2KB zero region; later slices rely
    on the pending-zero bits, which is exactly the HW semantics).
  - DVE: reciprocal of the denominator column + broadcast multiply, DMA out.

DMA: input loads are issued on both HWDGE queues (sync + scalar) to halve the
per-dma_start sequencer serialization; outputs go through gpsimd (SWDGE),
whose Q7 descriptor generator is otherwise idle.  qt is loaded as two
overlapping column chunks so the first matmul only waits for ~300KB.
"""

import os
import sys

for _p in ("/opt/trn_rl_repo", os.path.expanduser("~/.axon_site/_ro/trn_rl_repo")):
    if os.path.isdir(_p) and _p not in sys.path:
        sys.path.insert(0, _p)

import numpy as np

B, H, S, DK = 1, 16, 2048, 64
LOCAL_WINDOW = 256
N_CORES = 8
HPC = H // N_CORES  # heads per core
TB = 128            # tile block
NKB = S // TB       # key blocks per head
G = 4               # query blocks per psum output group
VW = DK + 1         # V columns + ones column
SPAN = LOCAL_WINDOW + TB          # 384: query span per key block
PAIRW = 2 * 512                   # paired st tile width (two psum banks)

QT_A_END = 1280                   # qt chunk A covers kb 0..6
QT_B_START = 896                  # qt chunk B covers kb 7..15

_prog_cache = {}


def _build_banded():
    import concourse.bass as bass
    import concourse.tile as tile
    from concourse import bacc, mybir

    fp16 = mybir.dt.float16
    fp32 = mybir.dt.float32

    nc = bacc.Bacc("TRN2", target_bir_lowering=False, debug=False)
    qt_d = nc.dram_tensor("qt", [TB, S], fp16, kind="ExternalInput")
    kt_d = nc.dram_tensor("kt", [TB, S], fp16, kind="ExternalInput")
    va_d = nc.dram_tensor("va", [TB, HPC * NKB * VW], fp16, kind="ExternalInput")
    mask_d = nc.dram_tensor("mask", [TB, 2 * SPAN], fp16, kind="ExternalInput")
    ctx_d = nc.dram_tensor("ctx", [HPC, S, DK], fp32, kind="ExternalOutput")

    with tile.TileContext(nc) as tc:
        with (
            tc.tile_pool(name="inp", bufs=1) as inp,
            tc.tile_pool(name="exp", bufs=2) as expp,
            tc.tile_pool(name="pt", bufs=3) as ptp,
            tc.tile_pool(name="stp", bufs=3, space="PSUM") as stp,
            tc.tile_pool(name="ctxp", bufs=2, space="PSUM") as ctxp,
            tc.tile_pool(name="outp", bufs=3) as outp,
        ):
            # ---- input loads, split across the two HWDGE queues ----
            kt_bounds = [0, 128, 512, 1024, 2048]
            kt_sb = []
            for c in range(len(kt_bounds) - 1):
                w = kt_bounds[c + 1] - kt_bounds[c]
                t = inp.tile([TB, w], fp16, tag=f"kt{c}", name=f"kt_sb{c}")
                kt_sb.append(t)
            va_sb = []
            va_cs = NKB * VW
            for c in range(HPC):
                t = inp.tile([TB, va_cs], fp16, tag=f"va{c}", name=f"va_sb{c}")
                va_sb.append(t)
            qt_sb = inp.tile([TB, S], fp16, tag="qt")
            mask_sb = inp.tile([TB, 2 * SPAN], fp16, tag="mask")

            # scalar queue: qt, mask ; sync queue: kt chunks + va
            nc.scalar.dma_start(qt_sb[:], qt_d.ap())
            nc.sync.dma_start(
                kt_sb[0][:], kt_d.ap()[:, kt_bounds[0]:kt_bounds[1]])
            nc.scalar.dma_start(mask_sb[:], mask_d.ap())
            nc.sync.dma_start(
                kt_sb[1][:], kt_d.ap()[:, kt_bounds[1]:kt_bounds[2]])
            nc.sync.dma_start(va_sb[0][:], va_d.ap()[:, 0:va_cs])

            def kt_slice(kb, hr):
                lo = kb * TB
                for c in range(len(kt_bounds) - 1):
                    if lo < kt_bounds[c + 1]:
                        return kt_sb[c][hr, lo - kt_bounds[c]:lo - kt_bounds[c] + TB]
                raise AssertionError

            def qt_slice(kb, hr, w):
                return qt_sb[hr, kb * TB:kb * TB + w]

            def va_slice(h, kb):
                return va_sb[h][:, kb * VW:(kb + 1) * VW]

            for h in range(HPC):
                hr = slice(h * DK, (h + 1) * DK)
                ctx_tiles = {}
                started = set()
                pt_tiles = {}
                for pair in range(NKB // 2):
                    kb0 = 2 * pair
                    # defer the non-critical bulk loads so their descriptors
                    # don't delay the first matmul's qt/kt0 arrival
                    if h == 0 and pair == 1:
                        nc.sync.dma_start(
                            kt_sb[2][:], kt_d.ap()[:, kt_bounds[2]:kt_bounds[3]])
                    if h == 0 and pair == 2:
                        nc.sync.dma_start(
                            kt_sb[3][:], kt_d.ap()[:, kt_bounds[3]:kt_bounds[4]])
                    if h == 0 and pair == 4:
                        nc.sync.dma_start(
                            va_sb[1][:], va_d.ap()[:, va_cs:2 * va_cs])
                    w0 = min(SPAN, S - kb0 * TB)
                    w1 = min(SPAN, S - (kb0 + 1) * TB)
                    st = stp.tile([TB, PAIRW], fp32, tag="st", name=f"st_{h}_{pair}")
                    nc.tensor.matmul(
                        st[:, 0:w0], lhsT=kt_slice(kb0, hr),
                        rhs=qt_slice(kb0, hr, w0), start=True, stop=True)
                    nc.tensor.matmul(
                        st[:, 512:512 + w1], lhsT=kt_slice(kb0 + 1, hr),
                        rhs=qt_slice(kb0 + 1, hr, w1), start=True, stop=True)
                    # exp reads both st banks through one 3D AP and writes a
                    # gap-free [128, 2*SPAN] fp16 tile
                    e = expp.tile([TB, 2 * SPAN], fp16, tag="exp",
                                  name=f"e_{h}_{pair}")
                    pt = ptp.tile([TB, 2 * SPAN], fp16, tag="pt",
                                  name=f"pt_{h}_{pair}")
                    if w0 == SPAN and w1 == SPAN:
                        st3 = st[:].rearrange("p (b c) -> p b c", c=512)[:, :, 0:SPAN]
                        e3 = e[:].rearrange("p (b c) -> p b c", c=SPAN)
                        nc.scalar.activation(
                            e3, st3, mybir.ActivationFunctionType.Exp)
                        nc.vector.tensor_mul(pt[:], e[:], mask_sb[:])
                    else:
                        nc.scalar.activation(
                            e[:, 0:w0], st[:, 0:w0],
                            mybir.ActivationFunctionType.Exp)
                        nc.vector.tensor_mul(
                            pt[:, 0:w0], e[:, 0:w0], mask_sb[:, 0:w0])
                        nc.scalar.activation(
                            e[:, SPAN:SPAN + w1], st[:, 512:512 + w1],
                            mybir.ActivationFunctionType.Exp)
                        nc.vector.tensor_mul(
                            pt[:, SPAN:SPAN + w1], e[:, SPAN:SPAN + w1],
                            mask_sb[:, SPAN:SPAN + w1])
                    pt_tiles[pair] = pt

                    # PV matmuls that become possible once this pair's P^T
                    # tiles exist
                    for qb in range(kb0, min(kb0 + 3, NKB - 1) + 1):
                        # contributions to qb available now: kb in
                        # [qb-2, qb] ∩ [.., kb0+1]; earlier kbs were done by
                        # previous pairs
                        for kb in range(max(0, qb - 2), min(qb, kb0 + 1) + 1):
                            if kb < kb0:
                                continue  # emitted by an earlier pair
                            g, j = divmod(qb, G)
                            if g not in ctx_tiles:
                                ctx_tiles[g] = ctxp.tile(
                                    [TB, G * VW], fp32, tag="ctx",
                                    name=f"ctx_{h}_{g}")
                            ct = ctx_tiles[g]
                            src = pt_tiles[kb // 2]
                            oo = (kb % 2) * SPAN + (qb - kb) * TB
                            last = (qb == g * G + G - 1) and (kb == qb)
                            nc.tensor.matmul(
                                ct[:, j * VW:(j + 1) * VW],
                                lhsT=src[:, oo:oo + TB],
                                rhs=va_slice(h, kb),
                                start=(g not in started), stop=last)
                            started.add(g)
                            if last:
                                ct3 = ct[:].rearrange("p (n c) -> p n c", c=VW)
                                recip = outp.tile([TB, G], fp32, tag="recip",
                                                  name=f"recip_{h}_{g}")
                                nc.vector.reciprocal(recip[:], ct3[:, :, DK])
                                out_sb = outp.tile([TB, G * DK], fp32, tag="out",
                                                   name=f"out_{h}_{g}")
                                out3 = out_sb[:].rearrange("p (n c) -> p n c", c=DK)
                                nc.vector.tensor_mul(
                                    out3, ct3[:, :, 0:DK],
                                    recip[:].unsqueeze(2).broadcast_to([TB, G, DK]))
                                dst = ctx_d.ap()[h, g * G * TB:(g + 1) * G * TB, :]
                                dst = dst.rearrange("(n p) d -> p n d", p=TB)
                                nc.sync.dma_start(dst, out3)
                                del ctx_tiles[g]
                                started.discard(g)
    nc.finalize()
    return nc


def _build_causal():
    """Correctness fallback for even layer_idx (full causal attention)."""
    import concourse.bass as bass
    import concourse.tile as tile
    from concourse import bacc, mybir

    fp16 = mybir.dt.float16
    fp32 = mybir.dt.float32
    win = S
    span_max = S
    mwidth = 512

    nc = bacc.Bacc("TRN2", target_bir_lowering=False, debug=False)
    qt_d = nc.dram_tensor("qt", [TB, S], fp16, kind="ExternalInput")
    kt_d = nc.dram_tensor("kt", [TB, S], fp16, kind="ExternalInput")
    va_d = nc.dram_tensor("va", [TB, HPC * NKB * VW], fp16, kind="ExternalInput")
    mask_d = nc.dram_tensor("mask", [TB, mwidth], fp16, kind="ExternalInput")
    ctx_d = nc.dram_tensor("ctx", [HPC, S, DK], fp32, kind="ExternalOutput")

    with tile.TileContext(nc) as tc:
        with (
            tc.tile_pool(name="inp", bufs=1) as inp,
            tc.tile_pool(name="exp", bufs=3) as expp,
            tc.tile_pool(name="pt", bufs=4) as ptp,
            tc.tile_pool(name="stp", bufs=2, space="PSUM") as stp,
            tc.tile_pool(name="ctxp", bufs=4, space="PSUM") as ctxp,
            tc.tile_pool(name="outp", bufs=3) as outp,
        ):
            mask_sb = inp.tile([TB, mwidth], fp16, tag="mask")
            nc.sync.dma_start(mask_sb[:], mask_d.ap())
            qt_sb = inp.tile([TB, S], fp16, tag="qt")
            nc.sync.dma_start(qt_sb[:], qt_d.ap())
            kt_sb = inp.tile([TB, S], fp16, tag="kt")
            nc.sync.dma_start(kt_sb[:], kt_d.ap())
            va_sb = inp.tile([TB, HPC * NKB * VW], fp16, tag="va")
            nc.sync.dma_start(va_sb[:], va_d.ap())

            for h in range(HPC):
                hr = slice(h * DK, (h + 1) * DK)
                ctx_tiles = {}
                started = set()
                for kb in range(NKB):
                    span = S - kb * TB
                    chunks = []
                    for o in range(0, span, 512):
                        w = min(512, span - o)
                        st = stp.tile([TB, 512], fp32, tag="st",
                                      name=f"st_{h}_{kb}_{o}")
                        nc.tensor.matmul(
                            st[:, 0:w], lhsT=kt_sb[hr, kb * TB:kb * TB + TB],
                            rhs=qt_sb[hr, kb * TB + o:kb * TB + o + w],
                            start=True, stop=True)
                        pt = ptp.tile([TB, 512], fp16, tag="pt",
                                      name=f"pt_{h}_{kb}_{o}")
                        if o == 0:
                            e = expp.tile([TB, 512], fp16, tag="exp",
                                          name=f"e_{h}_{kb}_{o}")
                            nc.scalar.activation(
                                e[:, 0:w], st[:, 0:w],
                                mybir.ActivationFunctionType.Exp)
                            nc.vector.tensor_mul(
                                pt[:, 0:w], e[:, 0:w], mask_sb[:, 0:w])
                        else:
                            nc.scalar.activation(
                                pt[:, 0:w], st[:, 0:w],
                                mybir.ActivationFunctionType.Exp)
                        chunks.append(pt)

                    for qb in range(kb, NKB):
                        g, j = divmod(qb, G)
                        if g not in ctx_tiles:
                            ctx_tiles[g] = ctxp.tile(
                                [TB, G * VW], fp32, tag="ctx", name=f"ctx_{h}_{g}")
                        ct = ctx_tiles[g]
                        o = (qb - kb) * TB
                        src = chunks[o // 512]
                        oo = o % 512
                        last = (qb == g * G + G - 1) and (kb == qb)
                        nc.tensor.matmul(
                            ct[:, j * VW:(j + 1) * VW],
                            lhsT=src[:, oo:oo + TB],
                            rhs=va_sb[:, (h * NKB + kb) * VW:(h * NKB + kb + 1) * VW],
                            start=(g not in started), stop=last)
                        started.add(g)
                        if last:
                            ct3 = ct[:].rearrange("p (n c) -> p n c", c=VW)
                            recip = outp.tile([TB, G], fp32, tag="recip",
                                              name=f"recip_{h}_{g}")
                            nc.vector.reciprocal(recip[:], ct3[:, :, DK])
                            out_sb = outp.tile([TB, G * DK], fp32, tag="out",
                                               name=f"out_{h}_{g}")
                            out3 = out_sb[:].rearrange("p (n c) -> p n c", c=DK)
                            nc.vector.tensor_mul(
                                out3, ct3[:, :, 0:DK],
                                recip[:].unsqueeze(2).broadcast_to([TB, G, DK]))
                            dst = ctx_d.ap()[h, g * G * TB:(g + 1) * G * TB, :]
                            dst = dst.rearrange("(n p) d -> p n d", p=TB)
                            nc.sync.dma_start(dst, out3)
                            del ctx_tiles[g]
                            started.discard(g)
    nc.finalize()
    return nc


def _get_program(win):
    if win not in _prog_cache:
        _prog_cache[win] = (
            _build_banded() if win == LOCAL_WINDOW else _build_causal())
    return _prog_cache[win]


def _make_mask_np(win):
    kl = np.arange(TB)[:, None]
    if win == LOCAL_WINDOW:
        m = np.zeros((TB, 2 * SPAN), np.float16)
        qs = np.arange(SPAN)[None, :]
        band = ((qs - kl) >= 0) & ((qs - kl) < win)
        m[:, 0:SPAN] = band
        m[:, SPAN:2 * SPAN] = band
        return m
    qs = np.arange(512)[None, :]
    return ((qs - kl) >= 0).astype(np.float16)


def make_in_maps(q, k, v, win):
    scale = np.float32(1.0 / np.sqrt(DK))
    mask_np = _make_mask_np(win)
    in_maps = []
    for c in range(N_CORES):
        heads = range(c * HPC, (c + 1) * HPC)
        qt = np.concatenate(
            [(q[0, h] * scale).T for h in heads], axis=0).astype(np.float16)
        kt = np.concatenate(
            [k[0, h].T for h in heads], axis=0).astype(np.float16)
        va = np.empty((TB, HPC * NKB * VW), np.float16)
        for hi, h in enumerate(heads):
            vh = np.concatenate(
                [v[0, h], np.ones((S, 1), np.float32)], axis=1)  # [S, 65]
            va[:, hi * NKB * VW:(hi + 1) * NKB * VW] = (
                vh.reshape(NKB, TB, VW).transpose(1, 0, 2).reshape(TB, NKB * VW)
            ).astype(np.float16)
        in_maps.append({
            "qt": np.ascontiguousarray(qt),
            "kt": np.ascontiguousarray(kt),
            "va": np.ascontiguousarray(va),
            "mask": mask_np,
        })
    return in_maps


def kernel(q, k, v, layer_idx=1, training=0):
    from concourse.bass_utils import run_bass_kernel_spmd

    q = np.asarray(q)
    k = np.asarray(k)
    v = np.asarray(v)
    li = int(np.asarray(layer_idx))
    win = S if li % 2 == 0 else LOCAL_WINDOW

    nc = _get_program(win)
    in_maps = make_in_maps(q, k, v, win)
    res = run_bass_kernel_spmd(nc, in_maps, core_ids=list(range(N_CORES)))

    ctx = np.empty((B, H, S, DK), np.float32)
    for c in range(N_CORES):
        out = res.results[c]["ctx"]  # [HPC, S, DK]
        for hi in range(HPC):
            ctx[0, c * HPC + hi] = out[hi]
    return ctx, k, v
